# revision 1
# baseline (speedup 1.0000x reference)
"""Trainium2 Bass kernel for nn_NONLocal_Correlation (non-local block, B=2,
C=CI=256, N=8192).

Sharding: 8 cores = (batch b, query-chunk q) with b = core//4, q = core%4.
Each core computes out[b, :, q*2048:(q+1)*2048]. x[b] is passed rolled by
-q*2048 along n so the core's query columns are always x_rot[:, :2048];
m-order permutation is irrelevant (softmax sums over m).

Per-core algorithm (all matmul operands float32r = fp32 bits, TF32-like PE
mode at full 1 row/cycle streaming rate; measured ~100% PE occupancy):
  - scores are algebraically reduced: f[n,m] = (th_w x_n + th_b).(ph_w x_m +
    ph_b) = x_n.(M x_m) + v.x_m + c_n with M = th_w.T @ ph_w and
    v = ph_w.T @ th_b host-precomputed; the n-only term c_n drops out of the
    softmax over m.  So only ONE projection u = M x is computed (no theta,
    no phi), and v.x_m rides as column CI of the widened g projection,
    applied as the exp's per-partition bias.  All biases thus cost nothing:
    g_b/w_b shift z per-channel and cancel against BatchNorm's mean.
  - w_w is also folded into the g weights on the host (z = w_w @ sum g E =
    sum (w_w g) E), so the attention's PSUM accumulator produces z directly
    and no separate z matmuls exist.
  - projections (u full, g' full, interleaved into chunk-0's m-loop with
    their PSUM-ring allocations spread between f-emissions) and attention,
    scores transposed (m on partitions): per 512-wide n-chunk, 64 m-blocks:
        f_T = u_blk.T @ x_chunk                 (PSUM)
        E   = exp(f_T + bias_m)                 (|f| < ~40, fp32-safe)
        z  += g'_blk.T @ E                      (PSUM accumulate, ci=2x128)
        s_acc += E                              (DVE; softmax denominator)
    The y-stream is emitted SKEW=2 m-blocks behind the f-stream so the PE
    never stalls on ACT's exp (keeps the 2.4GHz warm p-state).
  - chunk tails (ones-matmul denominator reduce, reciprocal, z *= rec, BN
    partial sum/sumsq) are spread into the next chunk's m-loop.
  - BatchNorm (training stats over (b, n)): per-core partial sum/sumsq per
    channel, 2KB AllReduce across all 8 cores.  Everything after the last
    y-matmul (final chunk tail, stats reduce + AllReduce launch, and the
    collective-consuming rsqrt/affine/writeback) is deferred into the NEXT
    replicated body's early emission so no in-order engine queue serializes
    the collective's latency at a body boundary; rsqrt is computed on DVE
    (bit-trick + Newton) to avoid ACT activation-table swaps.
"""

import numpy as np

import concourse.bacc as bacc
import concourse.mybir as mybir
import concourse.tile as tile
from concourse.bass_utils import run_bass_kernel_spmd

B, C, N, CI = 2, 256, 8192, 256
CIP = CI + 2              # g-projection moving width: g channels + exp-bias col
                          # + zero pad (fp32r matmul needs an even free dim)
CHUNK = N // 4            # 2048 query rows per core
W = 512                   # moving free-dim width
MB = N // 128             # 64 m-blocks
SW = 512                  # x strip width for projections
MB_PER_STRIP = SW // 128  # 4
EPS = 1e-5
NCORES = 8

F32 = mybir.dt.float32
F32R = mybir.dt.float32r
AF = mybir.ActivationFunctionType
AX = mybir.AxisListType


def build_body(nc, tc, pools, tensors, prev_steps=None):
    """Emit one full per-core computation. Separated so timing harnesses can
    replicate the body.

    Returns a list of deferred-tail closures (the post-AllReduce BN affine +
    writeback). The caller emits them inside the NEXT body's chunk-0 strip
    loop so the 2KB collective's latency overlaps the next body's compute
    instead of serializing every in-order engine queue at the body boundary;
    `prev_steps` is the previous body's list (emitted here), None for the
    first body."""
    wp, xp, pp, ep, sp, psf, psa, psz, dp = pools
    x, wts, consts, out, ones_in = tensors

    # --- first x strip prefetch (ahead of weights in the sync DMA queue) ---
    xs0 = []
    for hi in (0, 1):
        t = xp.tile([128, SW], F32R, name=f"xs{hi}", tag=f"xs{hi}")
        nc.sync.dma_start(t[:], x[hi * 128:(hi + 1) * 128, 0:SW])
        xs0.append(t)

    # --- weights / constants (u weights go on the scalar queue so they
    # don't queue behind the x strip on sync) ---
    w_sb = {}
    for p in ("ph", "g"):
        for hi in (0, 1):
            wd = CIP if p == "g" else CI
            t = wp.tile([128, wd], F32R, name=f"w_{p}{hi}", tag=f"w_{p}{hi}")
            (nc.scalar if p == "ph" else nc.sync).dma_start(t[:], wts[p][hi])
            w_sb[p, hi] = t
    # bufs=2: the previous body's deferred BN tail reads its own cst while
    # this body's cst DMA lands — single-buffered this WAR would chain the
    # body start behind the previous body's collective.
    cst = wp.tile([128, 10], F32, name="cst", tag="cst", bufs=2)
    nc.scalar.dma_start(cst[:], consts[:])
    ones = wp.tile([128, 128], F32R, name="ones", tag="ones")
    nc.scalar.dma_start(ones[:], ones_in[:])

    # --- persistent activations ---
    # phi[hi] holds u = M x with M = theta_w.T @ phi_w (host-precomputed):
    # scores f[n,m] = u_m . x_n + v . x_m (v = phi_w.T @ theta_b rides as
    # g-projection column CI, applied as the exp's per-partition bias), so no
    # theta projection is needed and the f-matmul's moving operand is raw x.
    phi = [pp.tile([128, N], F32R, name=f"phi{hi}", tag=f"phi{hi}") for hi in (0, 1)]
    gsb = pp.tile([128, MB * CIP], F32R, name="gsb", tag="gsb")
    zsb = [pp.tile([128, CHUNK], F32, name=f"z{hi}", tag=f"z{hi}") for hi in (0, 1)]
    # per-(nq,hi) BN partials: cols hi*4+nq sum, 8 + hi*4+nq sumsq
    parts = sp.tile([128, 16], F32, name="parts", tag="parts")

    xqs = {}

    def xq_fetch(nq):
        """Raw x columns for chunk nq's f-matmul moving operand (prefetched
        one chunk ahead; the bufs=2 ring also decouples body boundaries)."""
        ts = []
        for hi in (0, 1):
            t = xp.tile([128, W], F32R, name=f"xq{hi}", tag=f"xq{hi}", bufs=2)
            nc.sync.dma_start(t[:], x[hi * 128:(hi + 1) * 128,
                                      nq * W:(nq + 1) * W])
            ts.append(t)
        xqs[nq] = ts

    def fetch_xs(s):
        xs = []
        for hi in (0, 1):
            t = xp.tile([128, SW], F32R, name=f"xs{hi}", tag=f"xs{hi}")
            nc.sync.dma_start(t[:], x[hi * 128:(hi + 1) * 128,
                                      s * SW:(s + 1) * SW])
            xs.append(t)
        return xs

    def proj_u(s, xs, ho):
        fp = psf.tile([128, W], F32, name="mm_ps", tag="mm")
        nc.tensor.matmul(fp[:], w_sb["ph", 0][:, ho * 128:(ho + 1) * 128],
                         xs[0][:], start=True, stop=False)
        nc.tensor.matmul(fp[:], w_sb["ph", 1][:, ho * 128:(ho + 1) * 128],
                         xs[1][:], start=False, stop=True)
        nc.scalar.activation(phi[ho][:, s * SW:(s + 1) * SW], fp[:],
                             AF.Identity)

    def proj_g(s, xs, blk):
        mb = s * MB_PER_STRIP + blk
        bsl = slice(blk * 128, (blk + 1) * 128)
        gp = psf.tile([128, CIP], F32, name="g_ps", tag="mm")
        nc.tensor.matmul(gp[:], xs[0][:, bsl], w_sb["g", 0][:],
                         start=True, stop=False)
        nc.tensor.matmul(gp[:], xs[1][:, bsl], w_sb["g", 1][:],
                         start=False, stop=True)
        nc.vector.tensor_copy(gsb[:, mb * CIP:(mb + 1) * CIP], gp[:])

    def proj_strip(s, xs=None):
        if xs is None:
            xs = fetch_xs(s)
        for ho in (0, 1):
            proj_u(s, xs, ho)
        for blk in range(MB_PER_STRIP):
            proj_g(s, xs, blk)

    # attention state per n-chunk (created by att_begin)
    att = {}

    def att_begin(nq):
        att[nq] = {
            "yps": [psa.tile([128, W], F32, name=f"y_ps{hi}", tag=f"y_ps{hi}",
                             bufs=2) for hi in (0, 1)],
            "sacc": sp.tile([128, W], F32R, name="s_acc", tag="s_acc", bufs=2),
            "E": {},
        }

    def att_f(nq, mb):
        """Score matmuls + exp for one m-block (PE then ACT)."""
        msl = slice(mb * 128, (mb + 1) * 128)
        xq = xqs[nq]
        fp = psf.tile([128, W], F32, name="f_ps", tag="mm")
        nc.tensor.matmul(fp[:], phi[0][:, msl], xq[0][:],
                         start=True, stop=False)
        nc.tensor.matmul(fp[:], phi[1][:, msl], xq[1][:],
                         start=False, stop=True)
        E = ep.tile([128, W], F32R, name="E", tag="E")
        nc.scalar.activation(E[:], fp[:], AF.Exp,
                             bias=gsb[:, mb * CIP + CI:mb * CIP + CI + 1])
        att[nq]["E"][mb] = E

    def att_y(nq, mb):
        """y-accumulate matmuls + denominator add for one m-block. Emitted
        SKEW m-blocks behind att_f so the PE has f work in its queue while
        ACT's exp for this block completes (keeps the PE p-state warm)."""
        yps, sacc = att[nq]["yps"], att[nq]["sacc"]
        E = att[nq]["E"].pop(mb)
        st, fin = (mb == 0), (mb == MB - 1)
        nc.tensor.matmul(yps[0][:], gsb[:, mb * CIP:mb * CIP + 128],
                         E[:], start=st, stop=fin)
        nc.tensor.matmul(yps[1][:], gsb[:, mb * CIP + 128:mb * CIP + CI],
                         E[:], start=st, stop=fin)
        if mb == 0:
            nc.vector.tensor_copy(sacc[:], E[:])
        else:
            nc.vector.tensor_add(sacc[:], sacc[:], E[:])

    def tail_a(nq):
        """Chunk tail part 1: reduce+broadcast the softmax denominators with
        a single ones-matmul, then reciprocal (consumed by tail_z's muls)."""
        sacc = att[nq]["sacc"]
        s_red = psf.tile([128, W], F32, name="s_red", tag="mm")
        nc.tensor.matmul(s_red[:], ones[:], sacc[:], start=True, stop=True)
        rec = sp.tile([128, W], F32, name="rec", tag="rec", bufs=2)
        nc.vector.reciprocal(rec[:], s_red[:])
        att[nq]["rec"] = rec

    def tail_z(nq, hi):
        """Chunk tail for one ci-half: the y-accumulator IS z (w_w folded
        into the g weights on the host), so just normalize by the softmax
        denominator and take BN partials.
        No +zb bias either: BatchNorm subtracts the per-channel mean, so the
        per-channel z bias (w_w@g_b + w_b) cancels out of the final output."""
        nsl = slice(nq * W, (nq + 1) * W)
        rec = att[nq]["rec"]
        nc.vector.tensor_mul(zsb[hi][:, nsl], att[nq]["yps"][hi][:], rec[:])
        col = hi * 4 + nq
        nc.vector.reduce_sum(parts[:, col:col + 1], zsb[hi][:, nsl], axis=AX.X)
        sq = sp.tile([128, W], F32, name="sq_scr", tag="sq_scr", bufs=1)
        nc.scalar.activation(sq[:], zsb[hi][:, nsl], AF.Square,
                             accum_out=parts[:, 8 + col:9 + col])

    # --- emission: interleave projections into attention chunk 0; run the
    # y-stream SKEW m-blocks behind the f-stream (PE is in-order, so without
    # the skew every y waits ~the exp latency on ACT); each chunk's z-tail
    # goes into the next chunk's m-loop ---
    NQ = CHUNK // W
    SKEW = 2
    seq = [(nq, mb) for nq in range(NQ) for mb in range(MB)]
    state = {"fi": 0, "yi": 0}

    def emit_f():
        nq, mb = seq[state["fi"]]
        if mb == 0:
            att_begin(nq)
            if nq + 1 < NQ:
                xq_fetch(nq + 1)  # prefetch next chunk's x columns
        att_f(nq, mb)
        state["fi"] += 1

    def emit_y():
        nq, mb = seq[state["yi"]]
        att_y(nq, mb)
        state["yi"] += 1
        # previous chunk's tails, spread into this chunk so their
        # cross-engine producers (final sacc add on DVE, ysb copies) are done
        # before the PE reaches s_red / the z matmuls, and so the tail's ACT
        # work (Square+accum) fits into the exp stream's slack
        if nq >= 1 and mb == 1:
            tail_a(nq - 1)
        if nq >= 1 and mb == 5:
            tail_z(nq - 1, 0)
        if nq >= 1 and mb == 15:
            tail_z(nq - 1, 1)

    def fstep():
        emit_f()
        while state["fi"] - state["yi"] > SKEW:
            emit_y()

    xq_fetch(0)
    proj_strip(0, xs=xs0)  # first u/g blocks
    emit_f()               # (0, 0)
    for s in range(1, N // SW):
        xs = fetch_xs(s)
        for st, fn in prev_steps or ():
            if st == s:
                fn()  # previous body's deferred tail piece
        # interleave the strip's PSUM allocations between f-emissions so the
        # shared psf ring's WAR partners (ACT u-epilogue, DVE g-copy) get
        # ~2 PE slots of drain time instead of clustering. f(4s) stays after
        # both u-halves (it reads this strip's phi blocks).
        proj_u(s, xs, 0)
        proj_g(s, xs, 0)
        fstep()
        proj_g(s, xs, 1)
        fstep()
        proj_u(s, xs, 1)
        proj_g(s, xs, 2)
        fstep()
        proj_g(s, xs, 3)
        fstep()
    while state["fi"] < len(seq):
        emit_f()
        while state["fi"] - state["yi"] > SKEW:
            emit_y()
    while state["yi"] < len(seq):
        emit_y()

    # --- everything after the last y is deferred into the NEXT body's
    # chunk-0 strip loop (final chunk tails, BN stats reduce + AllReduce
    # launch, then the collective-consuming affine/writeback) so no engine
    # queue serializes at the body boundary ---
    statsg = sp.tile([128, 4], F32, name="statsg", tag="statsg")

    def bn_launch():
        tail_z(NQ - 1, 1)
        stats = sp.tile([128, 4], F32, name="stats", tag="stats")
        nc.vector.reduce_sum(stats[:],
                             parts[:].rearrange("p (g c) -> p g c", c=4),
                             axis=AX.X)
        ar_in = dp.tile([128, 4], F32, name="ar_in", tag="ar_in")
        ar_out = dp.tile([128, 4], F32, name="ar_out", tag="ar_out",
                         addr_space="Shared")
        nc.sync.dma_start(ar_in[:], stats[:])
        nc.gpsimd.collective_compute(
            "AllReduce", mybir.AluOpType.add,
            replica_groups=[list(range(NCORES))],
            ins=[ar_in.opt()], outs=[ar_out.opt()],
        )
        nc.gpsimd.dma_start(statsg[:], ar_out[:])

    box = {}

    def tail_stats():
        # All on DVE: an ACT Sqrt/Ln here would force a 1.3us activation-
        # table swap (and a swap back for Exp) in the middle of the next
        # body's exp stream. rsqrt via quake bit-trick + 2 Newton steps.
        inv_cnt = 1.0 / (B * N)
        moments = sp.tile([128, 4], F32, name="moments", tag="moments")
        nc.vector.tensor_scalar_mul(moments[:], statsg[:], inv_cnt)
        mean, ex2 = moments[:, 0:2], moments[:, 2:4]
        msq = sp.tile([128, 2], F32, name="msq", tag="msq")
        nc.vector.tensor_mul(msq[:], mean, mean)
        var = sp.tile([128, 2], F32, name="var", tag="var")
        nc.vector.tensor_sub(var[:], ex2, msq[:])
        vpe = sp.tile([128, 2], F32, name="vpe", tag="vpe")
        nc.vector.tensor_scalar_add(vpe[:], var[:], EPS)
        vh = sp.tile([128, 2], F32, name="vh", tag="vh")
        nc.vector.tensor_scalar_mul(vh[:], vpe[:], 0.5)
        I32 = mybir.dt.int32
        magic = sp.tile([128, 2], I32, name="magic", tag="magic")
        nc.vector.memset(magic[:], 0x5F3759DF)
        one_t = sp.tile([128, 2], I32, name="one_t", tag="one_t")
        nc.vector.memset(one_t[:], 1)
        rinv = sp.tile([128, 2], F32, name="rinv", tag="rinv")
        ri = rinv[:].bitcast(I32)
        nc.vector.tensor_tensor(ri, vpe[:].bitcast(I32), one_t[:],
                                mybir.AluOpType.logical_shift_right)
        nc.vector.tensor_tensor(ri, magic[:], ri, mybir.AluOpType.subtract)
        scr = sp.tile([128, 2], F32, name="scr", tag="scr")
        for _ in range(2):  # y *= 1.5 - 0.5*v*y^2
            nc.vector.tensor_mul(scr[:], rinv[:], rinv[:])
            nc.vector.tensor_mul(scr[:], scr[:], vh[:])
            nc.vector.tensor_scalar(scr[:], scr[:], -1.0, 1.5,
                                    mybir.AluOpType.mult, mybir.AluOpType.add)
            nc.vector.tensor_mul(rinv[:], rinv[:], scr[:])
        scl = sp.tile([128, 2], F32, name="scl", tag="scl")
        nc.vector.tensor_mul(scl[:], cst[:, 6:8], rinv[:])
        mscl = sp.tile([128, 2], F32, name="mscl", tag="mscl")
        nc.vector.tensor_mul(mscl[:], mean, scl[:])
        shf = sp.tile([128, 2], F32, name="shf", tag="shf")
        nc.vector.tensor_sub(shf[:], cst[:, 8:10], mscl[:])
        box["scl"], box["shf"] = scl, shf

    QTR = CHUNK // 4

    def make_affine(j):
        def step():
            scl, shf = box["scl"], box["shf"]
            jsl = slice(j * QTR, (j + 1) * QTR)
            nc.vector.tensor_scalar(zsb[0][:, jsl], zsb[0][:, jsl],
                                    scl[:, 0:1], shf[:, 0:1],
                                    mybir.AluOpType.mult, mybir.AluOpType.add)
            nc.scalar.activation(zsb[1][:, jsl], zsb[1][:, jsl], AF.Identity,
                                 bias=shf[:, 1:2], scale=scl[:, 1:2])
            nc.gpsimd.dma_start(out[0:128, jsl], zsb[0][:, jsl])
            nc.gpsimd.dma_start(out[128:256, jsl], zsb[1][:, jsl])
        return step

    return ([(1, lambda: tail_a(NQ - 1)),
             (2, lambda: tail_z(NQ - 1, 0)),
             (3, bn_launch),
             (11, tail_stats)] +
            [(12 + j, make_affine(j)) for j in range(4)])


def build_nc(n_bodies=1):
    nc = bacc.Bacc("TRN2", target_bir_lowering=False, debug=False,
                   num_devices=NCORES)
    x = nc.dram_tensor("x", [C, N], F32R, kind="ExternalInput")
    wts = {p: nc.dram_tensor(f"w_{p}", [2, 128, CIP if p == "g" else CI],
                             F32R, kind="ExternalInput")
           for p in ("ph", "g")}
    consts = nc.dram_tensor("consts", [128, 10], F32, kind="ExternalInput")
    ones_in = nc.dram_tensor("ones_in", [128, 128], F32R, kind="ExternalInput")
    out = nc.dram_tensor("out", [CI, CHUNK], F32, kind="ExternalOutput")

    with tile.TileContext(nc) as tc:
        with (
            tc.tile_pool(name="wp", bufs=1) as wp,
            tc.tile_pool(name="xp", bufs=2) as xp,
            tc.tile_pool(name="pp", bufs=1) as pp,
            tc.tile_pool(name="ep", bufs=4) as ep,
            tc.tile_pool(name="sp", bufs=1) as sp,
            tc.tile_pool(name="psf", bufs=4, space="PSUM") as psf,
            tc.tile_pool(name="psa", bufs=1, space="PSUM") as psa,
            tc.tile_pool(name="dp", bufs=1, space="DRAM") as dp,
        ):
            pools = (wp, xp, pp, ep, sp, psf, psa, None, dp)
            tensors = (x, wts, consts, out, ones_in)
            steps = None
            for _ in range(n_bodies):
                steps = build_body(nc, tc, pools, tensors, prev_steps=steps)
            for _, step in steps:  # last body's deferred tail
                step()
    nc.compile()
    return nc


def make_in_maps(inputs):
    x = np.asarray(inputs["x"], np.float32)
    theta_w = np.asarray(inputs["theta_w"], np.float64)
    phi_w = np.asarray(inputs["phi_w"], np.float64)
    theta_b = np.asarray(inputs["theta_b"], np.float64)
    # scores: f[n,m] = theta_x_n . phi_x_m = x_n.(M x_m) + v.x_m + c_n where
    # M = theta_w.T @ phi_w, v = phi_w.T @ theta_b; the n-only term c_n drops
    # out of the softmax over m. v rides as column CI of the g projection.
    M = theta_w.T @ phi_w
    v = phi_w.T @ theta_b
    # fold the z projection into g: z = w_w @ (sum_m g_m E_m) = sum_m
    # (w_w @ g_m) E_m, so the attention's y-accumulator produces z directly
    # and the separate z matmuls (and ysb copies) disappear. The exp-bias
    # column v stays unprojected (it feeds exp, not z).
    g_proj = (np.asarray(inputs["w_w"], np.float64)
              @ np.asarray(inputs["g_w"], np.float64))
    g_ext = np.concatenate(
        [g_proj.T.astype(np.float32),
         v[:, None].astype(np.float32),
         np.zeros((C, 1), np.float32)], axis=1)  # (C, CI+2)
    wT = {
        "ph": np.ascontiguousarray(M.T.astype(np.float32).reshape(2, 128, CI)),
        "g": np.ascontiguousarray(g_ext.reshape(2, 128, CIP)),
    }
    # note: no z bias needed anywhere — BatchNorm's mean subtraction absorbs
    # the per-channel w_w@g_b + w_b shift exactly
    consts = np.zeros((128, 10), np.float32)
    for j, vv in enumerate((np.zeros(CI), np.zeros(CI), np.zeros(CI),
                            inputs["bn_gamma"], inputs["bn_beta"])):
        vv = np.asarray(vv, np.float32)
        consts[:, 2 * j] = vv[:128]
        consts[:, 2 * j + 1] = vv[128:]
    in_maps = []
    for k in range(NCORES):
        b, q = divmod(k, 4)
        xb = np.roll(x[b], -q * CHUNK, axis=1)
        in_maps.append({
            "x": np.ascontiguousarray(xb),
            "w_ph": wT["ph"], "w_g": wT["g"],
            "consts": consts, "ones_in": np.ones((128, 128), np.float32),
        })
    return in_maps


def assemble(results):
    out = np.empty((B, CI, N), np.float32)
    for k in range(NCORES):
        b, q = divmod(k, 4)
        out[b, :, q * CHUNK:(q + 1) * CHUNK] = results[k]["out"]
    return out


_NC_CACHE = {}


def kernel(**inputs) -> np.ndarray:
    if "nc" not in _NC_CACHE:
        _NC_CACHE["nc"] = build_nc()
    nc = _NC_CACHE["nc"]
    in_maps = make_in_maps(inputs)
    res = run_bass_kernel_spmd(nc, in_maps, list(range(NCORES)))
    return assemble(res.results)



# revision 3
# speedup vs baseline: 1.2136x; 1.2136x over previous
"""Trainium2 Bass kernel for nn_NONLocal_Correlation (non-local block, B=2,
C=CI=256, N=8192).

Sharding: 8 cores = (batch b, query-chunk q) with b = core//4, q = core%4.
Each core computes out[b, :, q*2048:(q+1)*2048]. x[b] is passed rolled by
-q*2048 along n so the core's query columns are always x_rot[:, :2048];
m-order permutation is irrelevant (softmax sums over m).

Per-core algorithm — 16-bit matmul operands (fp16 for the score path, whose
error budget is tight; bf16 for the exp/value path, whose values need fp32
exponent range), fp32 PSUM accumulation throughout:
  - scores are algebraically reduced: f[n,m] = (th_w x_n + th_b).(ph_w x_m +
    ph_b) = x_n.(M x_m) + v.x_m + c_n with M = th_w.T @ ph_w and
    v = ph_w.T @ th_b host-precomputed; the n-only term c_n drops out of the
    softmax over m.  So only ONE projection u = M x is computed (no theta,
    no phi); v.x_m rides as column CI of the widened g projection and is
    extracted to an fp32 per-m-block bias tile for the exp.  All other
    biases cost nothing: g_b/w_b shift z per-channel and cancel against
    BatchNorm's mean subtraction.
  - w_w is folded into the g weights on the host (z = w_w @ sum g E =
    sum (w_w g) E), so the attention's PSUM accumulator produces z directly.
  - x is DMA'd once (fp16, strip-wise) and stays SBUF-resident; it serves as
    the projections' operand AND the f-matmul's moving operand.
  - attention per 512-wide n-chunk, 64 m-blocks, scores transposed (m on
    partitions):
        f_T = u_blk.T @ x_chunk                 (PSUM, fp16 operands)
        E   = exp(f_T + bias_m)                 (ACT, bf16 out; |f| < ~40)
        z  += g'_blk.T @ E                      (PSUM accumulate, bf16 ops)
        s_acc += E                              (DVE fp32; softmax denom)
    The y-stream is emitted SKEW=2 m-blocks behind the f-stream so the PE
    never stalls on ACT's exp.
  - chunk tails (ones-matmul denominator reduce, reciprocal, z *= rec, BN
    partial sum/sumsq) are spread into the next chunk's m-loop.
  - BatchNorm (training stats over (b, n)): per-core partial sum/sumsq per
    channel, 2KB AllReduce across all 8 cores.  Everything after the last
    y-matmul is deferred into the NEXT replicated body's early emission so
    no in-order engine queue serializes the collective's latency; rsqrt is
    computed on DVE (bit-trick + Newton) to avoid ACT table swaps.
"""

import numpy as np

import concourse.bacc as bacc
import concourse.mybir as mybir
import concourse.tile as tile
from concourse.bass_utils import run_bass_kernel_spmd

B, C, N, CI = 2, 256, 8192, 256
CIP = CI + 2              # g-projection moving width: g channels + exp-bias col
                          # + pad (even free dim)
CHUNK = N // 4            # 2048 query rows per core
W = 512                   # moving free-dim width
MB = N // 128             # 64 m-blocks
SW = 512                  # x strip width for projections
MB_PER_STRIP = SW // 128  # 4
EPS = 1e-5
NCORES = 8

F32 = mybir.dt.float32
F32R = mybir.dt.float32r
FDT = mybir.dt.float16    # score-path operand dtype (x, u, score weights)
YDT = mybir.dt.bfloat16   # value-path operand dtype (E, g')
AF = mybir.ActivationFunctionType
AX = mybir.AxisListType


def build_body(nc, tc, pools, tensors, prev_steps=None):
    """Emit one full per-core computation. Separated so timing harnesses can
    replicate the body.

    Returns a list of deferred-tail closures (the post-AllReduce BN affine +
    writeback) emitted inside the NEXT body's chunk-0 strip loop; see
    module docstring."""
    wp, xp, pp, ep, sp, psf, psa, psz, dp = pools
    x, wts, consts, out, ones_in = tensors

    # --- x halves, SBUF-resident, strip-wise DMA (strips 0-1 ahead of
    # weights in the sync queue; rest prefetched one strip ahead) ---
    xall = [xp.tile([128, N], FDT, name=f"xall{hi}", tag=f"xall{hi}")
            for hi in (0, 1)]
    fetched = {}

    def fetch_strip(s):
        if s in fetched:
            return
        fetched[s] = True
        for hi in (0, 1):
            nc.sync.dma_start(xall[hi][:, s * SW:(s + 1) * SW],
                              x[hi * 128:(hi + 1) * 128, s * SW:(s + 1) * SW])

    fetch_strip(0)
    fetch_strip(1)

    # --- weights / constants (u weights on the scalar queue so they don't
    # queue behind the x strips on sync) ---
    w_sb = {}
    for p in ("ph", "g"):
        for hi in (0, 1):
            wd = CIP if p == "g" else CI
            t = wp.tile([128, wd], FDT, name=f"w_{p}{hi}", tag=f"w_{p}{hi}")
            (nc.scalar if p == "ph" else nc.sync).dma_start(t[:], wts[p][hi])
            w_sb[p, hi] = t
    # bufs=2: the previous body's deferred BN tail reads its own cst while
    # this body's cst DMA lands.
    cst = wp.tile([128, 10], F32, name="cst", tag="cst", bufs=2)
    nc.scalar.dma_start(cst[:], consts[:])
    ones = wp.tile([128, 128], F32R, name="ones", tag="ones")
    nc.scalar.dma_start(ones[:], ones_in[:])

    # --- persistent activations ---
    # phi[hi] holds u = M x with M = theta_w.T @ phi_w (host-precomputed).
    phi = [pp.tile([128, N], FDT, name=f"phi{hi}", tag=f"phi{hi}") for hi in (0, 1)]
    gsb = pp.tile([128, MB * CIP], YDT, name="gsb", tag="gsb")
    biasb = pp.tile([128, MB], F32, name="biasb", tag="biasb")
    zsb = [pp.tile([128, CHUNK], F32, name=f"z{hi}", tag=f"z{hi}") for hi in (0, 1)]
    # per-(nq,hi) BN partials: cols hi*4+nq sum, 8 + hi*4+nq sumsq
    parts = sp.tile([128, 16], F32, name="parts", tag="parts")

    def proj_u(s, ho):
        fp = psf.tile([128, W], F32, name="mm_ps", tag="mm")
        nc.tensor.matmul(fp[:], w_sb["ph", 0][:, ho * 128:(ho + 1) * 128],
                         xall[0][:, s * SW:(s + 1) * SW], start=True, stop=False)
        nc.tensor.matmul(fp[:], w_sb["ph", 1][:, ho * 128:(ho + 1) * 128],
                         xall[1][:, s * SW:(s + 1) * SW], start=False, stop=True)
        nc.scalar.activation(phi[ho][:, s * SW:(s + 1) * SW], fp[:],
                             AF.Identity)

    def proj_g(s, blk):
        mb = s * MB_PER_STRIP + blk
        bsl = slice(s * SW + blk * 128, s * SW + (blk + 1) * 128)
        gp = psf.tile([128, CIP], F32, name="g_ps", tag="mm")
        nc.tensor.matmul(gp[:], xall[0][:, bsl], w_sb["g", 0][:],
                         start=True, stop=False)
        nc.tensor.matmul(gp[:], xall[1][:, bsl], w_sb["g", 1][:],
                         start=False, stop=True)
        nc.vector.tensor_copy(gsb[:, mb * CIP:(mb + 1) * CIP], gp[:])
        nc.vector.tensor_copy(biasb[:, mb:mb + 1], gp[:, CI:CI + 1])

    # attention state per n-chunk (created by att_begin)
    att = {}

    def att_begin(nq):
        att[nq] = {
            "yps": [psa.tile([128, W], F32, name=f"y_ps{hi}", tag=f"y_ps{hi}",
                             bufs=2) for hi in (0, 1)],
            "sacc": sp.tile([128, W], F32R, name="s_acc", tag="s_acc", bufs=2),
            "E": {},
        }

    def att_f(nq, mb):
        """Score matmuls + exp for one m-block (PE then ACT)."""
        msl = slice(mb * 128, (mb + 1) * 128)
        nsl = slice(nq * W, (nq + 1) * W)
        fp = psf.tile([128, W], F32, name="f_ps", tag="mm")
        nc.tensor.matmul(fp[:], phi[0][:, msl], xall[0][:, nsl],
                         start=True, stop=False)
        nc.tensor.matmul(fp[:], phi[1][:, msl], xall[1][:, nsl],
                         start=False, stop=True)
        E = ep.tile([128, W], YDT, name="E", tag="E")
        nc.scalar.activation(E[:], fp[:], AF.Exp,
                             bias=biasb[:, mb:mb + 1])
        att[nq]["E"][mb] = E

    def att_y(nq, mb):
        """y-accumulate matmuls + denominator add for one m-block. Emitted
        SKEW m-blocks behind att_f so the PE has f work queued while ACT's
        exp for this block completes."""
        yps, sacc = att[nq]["yps"], att[nq]["sacc"]
        E = att[nq]["E"].pop(mb)
        st, fin = (mb == 0), (mb == MB - 1)
        nc.tensor.matmul(yps[0][:], gsb[:, mb * CIP:mb * CIP + 128],
                         E[:], start=st, stop=fin)
        nc.tensor.matmul(yps[1][:], gsb[:, mb * CIP + 128:mb * CIP + CI],
                         E[:], start=st, stop=fin)
        if mb == 0:
            nc.vector.tensor_copy(sacc[:], E[:])
        else:
            nc.vector.tensor_add(sacc[:], sacc[:], E[:])

    def tail_a(nq):
        """Chunk tail part 1: reduce+broadcast the softmax denominators with
        a single ones-matmul, then reciprocal (consumed by tail_z's muls)."""
        sacc = att[nq]["sacc"]
        s_red = psf.tile([128, W], F32, name="s_red", tag="mm")
        nc.tensor.matmul(s_red[:], ones[:], sacc[:],
                         start=True, stop=True)
        rec = sp.tile([128, W], F32, name="rec", tag="rec", bufs=2)
        nc.vector.reciprocal(rec[:], s_red[:])
        att[nq]["rec"] = rec

    def tail_z(nq, hi):
        """Chunk tail for one ci-half: the y-accumulator IS z (w_w folded
        into the g weights on the host), so just normalize by the softmax
        denominator and take BN partials.
        No +zb bias either: BatchNorm subtracts the per-channel mean, so the
        per-channel z bias (w_w@g_b + w_b) cancels out of the final output."""
        nsl = slice(nq * W, (nq + 1) * W)
        rec = att[nq]["rec"]
        nc.vector.tensor_mul(zsb[hi][:, nsl], att[nq]["yps"][hi][:], rec[:])
        col = hi * 4 + nq
        nc.vector.reduce_sum(parts[:, col:col + 1], zsb[hi][:, nsl], axis=AX.X)
        sq = sp.tile([128, W], F32, name="sq_scr", tag="sq_scr", bufs=1)
        nc.scalar.activation(sq[:], zsb[hi][:, nsl], AF.Square,
                             accum_out=parts[:, 8 + col:9 + col])

    # --- emission: interleave projections into attention chunk 0; run the
    # y-stream SKEW m-blocks behind the f-stream; each chunk's z-tail goes
    # into the next chunk's m-loop ---
    NQ = CHUNK // W
    SKEW = 2
    seq = [(nq, mb) for nq in range(NQ) for mb in range(MB)]
    state = {"fi": 0, "yi": 0}

    def emit_f():
        nq, mb = seq[state["fi"]]
        if mb == 0:
            att_begin(nq)
        att_f(nq, mb)
        state["fi"] += 1

    def emit_y():
        nq, mb = seq[state["yi"]]
        att_y(nq, mb)
        state["yi"] += 1
        # previous chunk's tails, spread into this chunk so their
        # cross-engine producers are done before the PE reaches s_red / the
        # z matmuls, and so the tail's ACT work fits the exp stream's slack
        if nq >= 1 and mb == 1:
            tail_a(nq - 1)
        if nq >= 1 and mb == 5:
            tail_z(nq - 1, 0)
        if nq >= 1 and mb == 15:
            tail_z(nq - 1, 1)

    def fstep():
        emit_f()
        while state["fi"] - state["yi"] > SKEW:
            emit_y()

    # strip 0 projections, then the interleaved strip loop
    for ho in (0, 1):
        proj_u(0, ho)
    for blk in range(MB_PER_STRIP):
        proj_g(0, blk)
    emit_f()               # (0, 0)
    for s in range(1, N // SW):
        fetch_strip(s)
        if s + 1 < N // SW:
            fetch_strip(s + 1)  # prefetch next strip
        for st, fn in prev_steps or ():
            if st == s:
                fn()  # previous body's deferred tail piece
        # interleave the strip's PSUM allocations between f-emissions so the
        # shared psf ring's WAR partners (ACT u-epilogue, DVE g-copy) get
        # ~2 PE slots of drain time instead of clustering. f(4s) stays after
        # both u-halves (it reads this strip's phi blocks).
        proj_u(s, 0)
        proj_g(s, 0)
        fstep()
        proj_g(s, 1)
        fstep()
        proj_u(s, 1)
        proj_g(s, 2)
        fstep()
        proj_g(s, 3)
        fstep()
    while state["fi"] < len(seq):
        emit_f()
        while state["fi"] - state["yi"] > SKEW:
            emit_y()
    while state["yi"] < len(seq):
        emit_y()

    # --- everything after the last y is deferred into the NEXT body's
    # chunk-0 strip loop (final chunk tails, BN stats reduce + AllReduce
    # launch, then the collective-consuming affine/writeback) so no engine
    # queue serializes at the body boundary ---
    statsg = sp.tile([128, 4], F32, name="statsg", tag="statsg")

    def bn_launch():
        tail_z(NQ - 1, 1)
        stats = sp.tile([128, 4], F32, name="stats", tag="stats")
        nc.vector.reduce_sum(stats[:],
                             parts[:].rearrange("p (g c) -> p g c", c=4),
                             axis=AX.X)
        ar_in = dp.tile([128, 4], F32, name="ar_in", tag="ar_in")
        ar_out = dp.tile([128, 4], F32, name="ar_out", tag="ar_out",
                         addr_space="Shared")
        nc.sync.dma_start(ar_in[:], stats[:])
        nc.gpsimd.collective_compute(
            "AllReduce", mybir.AluOpType.add,
            replica_groups=[list(range(NCORES))],
            ins=[ar_in.opt()], outs=[ar_out.opt()],
        )
        nc.gpsimd.dma_start(statsg[:], ar_out[:])

    box = {}

    def tail_stats():
        # All on DVE: an ACT Sqrt/Ln here would force a 1.3us activation-
        # table swap in the middle of the next body's exp stream. rsqrt via
        # quake bit-trick + 2 Newton steps.
        inv_cnt = 1.0 / (B * N)
        moments = sp.tile([128, 4], F32, name="moments", tag="moments")
        nc.vector.tensor_scalar_mul(moments[:], statsg[:], inv_cnt)
        mean, ex2 = moments[:, 0:2], moments[:, 2:4]
        msq = sp.tile([128, 2], F32, name="msq", tag="msq")
        nc.vector.tensor_mul(msq[:], mean, mean)
        var = sp.tile([128, 2], F32, name="var", tag="var")
        nc.vector.tensor_sub(var[:], ex2, msq[:])
        vpe = sp.tile([128, 2], F32, name="vpe", tag="vpe")
        nc.vector.tensor_scalar_add(vpe[:], var[:], EPS)
        vh = sp.tile([128, 2], F32, name="vh", tag="vh")
        nc.vector.tensor_scalar_mul(vh[:], vpe[:], 0.5)
        I32 = mybir.dt.int32
        magic = sp.tile([128, 2], I32, name="magic", tag="magic")
        nc.vector.memset(magic[:], 0x5F3759DF)
        one_t = sp.tile([128, 2], I32, name="one_t", tag="one_t")
        nc.vector.memset(one_t[:], 1)
        rinv = sp.tile([128, 2], F32, name="rinv", tag="rinv")
        ri = rinv[:].bitcast(I32)
        nc.vector.tensor_tensor(ri, vpe[:].bitcast(I32), one_t[:],
                                mybir.AluOpType.logical_shift_right)
        nc.vector.tensor_tensor(ri, magic[:], ri, mybir.AluOpType.subtract)
        scr = sp.tile([128, 2], F32, name="scr", tag="scr")
        for _ in range(2):  # y *= 1.5 - 0.5*v*y^2
            nc.vector.tensor_mul(scr[:], rinv[:], rinv[:])
            nc.vector.tensor_mul(scr[:], scr[:], vh[:])
            nc.vector.tensor_scalar(scr[:], scr[:], -1.0, 1.5,
                                    mybir.AluOpType.mult, mybir.AluOpType.add)
            nc.vector.tensor_mul(rinv[:], rinv[:], scr[:])
        scl = sp.tile([128, 2], F32, name="scl", tag="scl")
        nc.vector.tensor_mul(scl[:], cst[:, 6:8], rinv[:])
        mscl = sp.tile([128, 2], F32, name="mscl", tag="mscl")
        nc.vector.tensor_mul(mscl[:], mean, scl[:])
        shf = sp.tile([128, 2], F32, name="shf", tag="shf")
        nc.vector.tensor_sub(shf[:], cst[:, 8:10], mscl[:])
        box["scl"], box["shf"] = scl, shf

    QTR = CHUNK // 4

    def make_affine(j):
        def step():
            scl, shf = box["scl"], box["shf"]
            jsl = slice(j * QTR, (j + 1) * QTR)
            nc.vector.tensor_scalar(zsb[0][:, jsl], zsb[0][:, jsl],
                                    scl[:, 0:1], shf[:, 0:1],
                                    mybir.AluOpType.mult, mybir.AluOpType.add)
            nc.scalar.activation(zsb[1][:, jsl], zsb[1][:, jsl], AF.Identity,
                                 bias=shf[:, 1:2], scale=scl[:, 1:2])
            nc.gpsimd.dma_start(out[0:128, jsl], zsb[0][:, jsl])
            nc.gpsimd.dma_start(out[128:256, jsl], zsb[1][:, jsl])
        return step

    return ([(1, lambda: tail_a(NQ - 1)),
             (2, lambda: tail_z(NQ - 1, 0)),
             (3, bn_launch),
             (11, tail_stats)] +
            [(12 + j, make_affine(j)) for j in range(4)])


def build_nc(n_bodies=1):
    nc = bacc.Bacc("TRN2", target_bir_lowering=False, debug=False,
                   num_devices=NCORES)
    x = nc.dram_tensor("x", [C, N], FDT, kind="ExternalInput")
    wts = {p: nc.dram_tensor(f"w_{p}", [2, 128, CIP if p == "g" else CI],
                             FDT, kind="ExternalInput")
           for p in ("ph", "g")}
    consts = nc.dram_tensor("consts", [128, 10], F32, kind="ExternalInput")
    ones_in = nc.dram_tensor("ones_in", [128, 128], F32R, kind="ExternalInput")
    out = nc.dram_tensor("out", [CI, CHUNK], F32, kind="ExternalOutput")

    with tile.TileContext(nc) as tc:
        with (
            tc.tile_pool(name="wp", bufs=1) as wp,
            tc.tile_pool(name="xp", bufs=2) as xp,
            tc.tile_pool(name="pp", bufs=1) as pp,
            tc.tile_pool(name="ep", bufs=4) as ep,
            tc.tile_pool(name="sp", bufs=1) as sp,
            tc.tile_pool(name="psf", bufs=4, space="PSUM") as psf,
            tc.tile_pool(name="psa", bufs=1, space="PSUM") as psa,
            tc.tile_pool(name="dp", bufs=1, space="DRAM") as dp,
        ):
            pools = (wp, xp, pp, ep, sp, psf, psa, None, dp)
            tensors = (x, wts, consts, out, ones_in)
            steps = None
            for _ in range(n_bodies):
                steps = build_body(nc, tc, pools, tensors, prev_steps=steps)
            for _, step in steps:  # last body's deferred tail
                step()
    nc.compile()
    return nc


def make_in_maps(inputs):
    np16 = mybir.dt.np(FDT)
    x = np.asarray(inputs["x"], np.float32)
    theta_w = np.asarray(inputs["theta_w"], np.float64)
    phi_w = np.asarray(inputs["phi_w"], np.float64)
    theta_b = np.asarray(inputs["theta_b"], np.float64)
    # scores: f[n,m] = theta_x_n . phi_x_m = x_n.(M x_m) + v.x_m + c_n where
    # M = theta_w.T @ phi_w, v = phi_w.T @ theta_b; the n-only term c_n drops
    # out of the softmax over m. v rides as column CI of the g projection.
    M = theta_w.T @ phi_w
    v = phi_w.T @ theta_b
    # fold the z projection into g: z = w_w @ (sum_m g_m E_m) = sum_m
    # (w_w @ g_m) E_m, so the attention's y-accumulator produces z directly.
    g_proj = (np.asarray(inputs["w_w"], np.float64)
              @ np.asarray(inputs["g_w"], np.float64))
    g_ext = np.concatenate(
        [g_proj.T.astype(np.float32),
         v[:, None].astype(np.float32),
         np.zeros((C, 1), np.float32)], axis=1)  # (C, CI+2)
    wT = {
        "ph": np.ascontiguousarray(M.T.astype(np16).reshape(2, 128, CI)),
        "g": np.ascontiguousarray(g_ext.astype(np16).reshape(2, 128, CIP)),
    }
    # note: no z bias needed anywhere — BatchNorm's mean subtraction absorbs
    # the per-channel w_w@g_b + w_b shift exactly
    consts = np.zeros((128, 10), np.float32)
    for j, vv in enumerate((np.zeros(CI), np.zeros(CI), np.zeros(CI),
                            inputs["bn_gamma"], inputs["bn_beta"])):
        vv = np.asarray(vv, np.float32)
        consts[:, 2 * j] = vv[:128]
        consts[:, 2 * j + 1] = vv[128:]
    in_maps = []
    for k in range(NCORES):
        b, q = divmod(k, 4)
        xb = np.roll(x[b], -q * CHUNK, axis=1)
        in_maps.append({
            "x": np.ascontiguousarray(xb.astype(np16)),
            "w_ph": wT["ph"], "w_g": wT["g"],
            "consts": consts, "ones_in": np.ones((128, 128), np.float32),
        })
    return in_maps


def assemble(results):
    out = np.empty((B, CI, N), np.float32)
    for k in range(NCORES):
        b, q = divmod(k, 4)
        out[b, :, q * CHUNK:(q + 1) * CHUNK] = results[k]["out"]
    return out


_NC_CACHE = {}


def kernel(**inputs) -> np.ndarray:
    if "nc" not in _NC_CACHE:
        _NC_CACHE["nc"] = build_nc()
    nc = _NC_CACHE["nc"]
    in_maps = make_in_maps(inputs)
    res = run_bass_kernel_spmd(nc, in_maps, list(range(NCORES)))
    return assemble(res.results)


# revision 5
# speedup vs baseline: 1.2501x; 1.0301x over previous
"""Trainium2 Bass kernel for nn_NONLocal_Correlation (non-local block, B=2,
C=CI=256, N=8192).

Sharding: 8 cores = (batch b, query-chunk q) with b = core//4, q = core%4.
Each core computes out[b, :, q*2048:(q+1)*2048]. x[b] is passed rolled by
-q*2048 along n so the core's query columns are always x_rot[:, :2048];
m-order permutation is irrelevant (softmax sums over m).

Per-core algorithm — 16-bit matmul operands (fp16 for the score path, whose
error budget is tight; bf16 for the exp/value path, whose values need fp32
exponent range), fp32 PSUM accumulation throughout:
  - scores are algebraically reduced: f[n,m] = (th_w x_n + th_b).(ph_w x_m +
    ph_b) = x_n.(M x_m) + v.x_m + c_n with M = th_w.T @ ph_w and
    v = ph_w.T @ th_b host-precomputed; the n-only term c_n drops out of the
    softmax over m.  So only ONE projection u = M x is computed; v.x_m rides
    as column CI of the widened g projection and is extracted to an fp32
    per-m-block bias tile for the exp.  Other biases cost nothing: g_b/w_b
    shift z per-channel and cancel against BatchNorm's mean subtraction.
  - w_w is folded into the g weights on the host (z = w_w @ sum g E =
    sum (w_w g) E), so the attention's PSUM accumulator produces z directly.
  - x is DMA'd once (fp16, strip-wise) and stays SBUF-resident; it serves as
    the projections' operand AND the f-matmul's moving operand.
  - attention processes n-chunks in PAIRS (A, B) so each stationary operand
    serves two moving streams back-to-back; legalization emits an
    InstLdweights before every matmul, and `_dedupe_ldweights` removes the
    now-redundant consecutive identical loads post-compile (validated
    bit-exact on HW), halving the PE's weight-load overhead:
        f_A += u_blk.T x_A ; f_B += u_blk.T x_B      (ldw u0,[u0],u1,[u1])
        E_A = exp(f_A + bias_m); E_B likewise        (ACT, bf16 out)
        z_A += g_blk.T E_A ; z_B += g_blk.T E_B      (ldw g0,[g0],g1,[g1])
        s_acc_A += E_A ; s_acc_B += E_B              (DVE fp32 denominator)
    The y-stream runs SKEW=2 m-blocks behind the f-stream so the PE never
    stalls on ACT's exp.  PSUM: 4 y-accumulator banks (bufs=1, reused by the
    next pair after its tails) + the shared 4-bank matmul ring.
  - pair tails (ones-matmul denominator reduce, reciprocal, z *= rec, BN
    partials) are emitted at the next pair's head — before its first
    y-allocation so the bufs=1 PSUM WAR ordering holds.
  - BatchNorm (training stats over (b, n)): per-core partial sum/sumsq per
    channel, 2KB AllReduce across all 8 cores.  Everything after the last
    y-matmul is deferred into the NEXT replicated body's strip-1 emission so
    no in-order engine queue serializes the collective's latency; rsqrt is
    computed on DVE (bit-trick + Newton) to avoid ACT table swaps.
"""

import numpy as np

import concourse.bacc as bacc
import concourse.mybir as mybir
import concourse.tile as tile
from concourse.bass_utils import run_bass_kernel_spmd

B, C, N, CI = 2, 256, 8192, 256
CIP = CI + 2              # g-projection moving width: g channels + exp-bias col
                          # + pad (even free dim)
CHUNK = N // 4            # 2048 query rows per core
W = 512                   # moving free-dim width
MB = N // 128             # 64 m-blocks
SW = 512                  # x strip width for projections
MB_PER_STRIP = SW // 128  # 4
EPS = 1e-5
NCORES = 8

F32 = mybir.dt.float32
F32R = mybir.dt.float32r
FDT = mybir.dt.float16    # score-path operand dtype (x, u, score weights)
YDT = mybir.dt.bfloat16   # value-path operand dtype (E, g')
AF = mybir.ActivationFunctionType
AX = mybir.AxisListType

PAIRS = ((0, 1), (2, 3))  # n-chunk pairs sharing stationary loads


def build_body(nc, tc, pools, tensors, prev_steps=None):
    """Emit one full per-core computation. Separated so timing harnesses can
    replicate the body.

    Returns a list of deferred-tail closures (pair-1 tails + BN collective +
    affine/writeback) emitted inside the NEXT body's strip loop; see module
    docstring."""
    wp, xp, pp, ep, sp, psf, psa, psz, dp = pools
    x, wts, consts, out, ones_in = tensors

    # --- x halves, SBUF-resident, strip-wise DMA (strips 0-1 ahead of
    # weights in the sync queue; rest prefetched one strip ahead) ---
    xall = [xp.tile([128, N], FDT, name=f"xall{hi}", tag=f"xall{hi}")
            for hi in (0, 1)]
    fetched = {}

    def fetch_strip(s):
        if s in fetched:
            return
        fetched[s] = True
        for hi in (0, 1):
            nc.sync.dma_start(xall[hi][:, s * SW:(s + 1) * SW],
                              x[hi * 128:(hi + 1) * 128, s * SW:(s + 1) * SW])

    fetch_strip(0)
    fetch_strip(1)

    # --- weights / constants (u weights on the scalar queue so they don't
    # queue behind the x strips on sync) ---
    w_sb = {}
    for p in ("ph", "g"):
        for hi in (0, 1):
            wd = CIP if p == "g" else CI
            t = wp.tile([128, wd], FDT, name=f"w_{p}{hi}", tag=f"w_{p}{hi}")
            (nc.scalar if p == "ph" else nc.sync).dma_start(t[:], wts[p][hi])
            w_sb[p, hi] = t
    # bufs=2: the previous body's deferred BN tail reads its own cst while
    # this body's cst DMA lands.
    cst = wp.tile([128, 10], F32, name="cst", tag="cst", bufs=2)
    nc.scalar.dma_start(cst[:], consts[:])
    ones = wp.tile([128, 128], F32R, name="ones", tag="ones")
    nc.scalar.dma_start(ones[:], ones_in[:])

    # --- persistent activations ---
    # phi[hi] holds u = M x with M = theta_w.T @ phi_w (host-precomputed).
    phi = [pp.tile([128, N], FDT, name=f"phi{hi}", tag=f"phi{hi}") for hi in (0, 1)]
    gsb = pp.tile([128, MB * CIP], YDT, name="gsb", tag="gsb")
    biasb = pp.tile([128, MB], F32, name="biasb", tag="biasb")
    zsb = [pp.tile([128, CHUNK], F32, name=f"z{hi}", tag=f"z{hi}") for hi in (0, 1)]
    # per-(nq,hi) BN partials: cols hi*4+nq sum, 8 + hi*4+nq sumsq
    parts = sp.tile([128, 16], F32, name="parts", tag="parts")

    def proj_u(s, ho):
        fp = psf.tile([128, W], F32, name="mm_ps", tag="mm")
        nc.tensor.matmul(fp[:], w_sb["ph", 0][:, ho * 128:(ho + 1) * 128],
                         xall[0][:, s * SW:(s + 1) * SW], start=True, stop=False)
        nc.tensor.matmul(fp[:], w_sb["ph", 1][:, ho * 128:(ho + 1) * 128],
                         xall[1][:, s * SW:(s + 1) * SW], start=False, stop=True)
        nc.scalar.activation(phi[ho][:, s * SW:(s + 1) * SW], fp[:],
                             AF.Identity)

    def proj_g(s, blk):
        mb = s * MB_PER_STRIP + blk
        bsl = slice(s * SW + blk * 128, s * SW + (blk + 1) * 128)
        gp = psf.tile([128, CIP], F32, name="g_ps", tag="mm")
        nc.tensor.matmul(gp[:], xall[0][:, bsl], w_sb["g", 0][:],
                         start=True, stop=False)
        nc.tensor.matmul(gp[:], xall[1][:, bsl], w_sb["g", 1][:],
                         start=False, stop=True)
        nc.vector.tensor_copy(gsb[:, mb * CIP:(mb + 1) * CIP], gp[:])
        nc.vector.tensor_copy(biasb[:, mb:mb + 1], gp[:, CI:CI + 1])

    # attention state per chunk-pair. The f-stream only needs the E dict
    # (att_begin_f); the PSUM y-accumulators are allocated lazily at the
    # pair's first y-matmul (att_begin_y) so the previous pair's tails are
    # emitted first — the bufs=1 bank reuse WAR needs that order.
    att = {}

    def att_begin_f(p):
        att[p] = {"yps": None, "sacc": None, "rec": {}, "E": {}}

    def att_begin_y(p):
        att[p]["yps"] = {(ci, hi): psa.tile([128, W], F32,
                                            name=f"y_ps{ci}{hi}",
                                            tag=f"y_ps{ci}{hi}", bufs=1)
                         for ci in (0, 1) for hi in (0, 1)}
        att[p]["sacc"] = {ci: sp.tile([128, W], F32R, name=f"s_acc{ci}",
                                      tag=f"s_acc{ci}", bufs=2)
                          for ci in (0, 1)}

    def att_f(p, mb):
        """Score matmuls + exps for one m-block of both pair members.
        Emission order phi0:A, phi0:B, phi1:A, phi1:B makes the B-loads
        redundant for the post-compile ldweights dedupe."""
        msl = slice(mb * 128, (mb + 1) * 128)
        nsls = [slice(nq * W, (nq + 1) * W) for nq in PAIRS[p]]
        fps = [psf.tile([128, W], F32, name=f"f_ps{ci}", tag="mm")
               for ci in (0, 1)]
        for hi in (0, 1):
            for ci in (0, 1):
                nc.tensor.matmul(fps[ci][:], phi[hi][:, msl],
                                 xall[hi][:, nsls[ci]],
                                 start=(hi == 0), stop=(hi == 1))
        Es = []
        for ci in (0, 1):
            E = ep.tile([128, W], YDT, name=f"E{ci}", tag=f"E{ci}")
            nc.scalar.activation(E[:], fps[ci][:], AF.Exp,
                                 bias=biasb[:, mb:mb + 1])
            Es.append(E)
        att[p]["E"][mb] = Es

    def att_y(p, mb):
        """y-accumulate matmuls + denominator adds for one m-block, both
        pair members. Emitted SKEW m-blocks behind att_f. Order g0:A, g0:B,
        g1:A, g1:B for the ldweights dedupe."""
        yps, sacc = att[p]["yps"], att[p]["sacc"]
        Es = att[p]["E"].pop(mb)
        st, fin = (mb == 0), (mb == MB - 1)
        for hi in (0, 1):
            gs = gsb[:, mb * CIP + hi * 128:mb * CIP + (hi + 1) * 128]
            for ci in (0, 1):
                nc.tensor.matmul(yps[ci, hi][:], gs, Es[ci][:],
                                 start=st, stop=fin)
        for ci in (0, 1):
            if mb == 0:
                nc.vector.tensor_copy(sacc[ci][:], Es[ci][:])
            else:
                nc.vector.tensor_add(sacc[ci][:], sacc[ci][:], Es[ci][:])

    def tail_a(p, ci):
        """Pair tail part 1 for one member: reduce+broadcast the softmax
        denominators with a ones-matmul, then reciprocal."""
        sacc = att[p]["sacc"][ci]
        s_red = psf.tile([128, W], F32, name="s_red", tag="mm")
        nc.tensor.matmul(s_red[:], ones[:], sacc[:], start=True, stop=True)
        rec = sp.tile([128, W], F32, name="rec", tag="rec", bufs=2)
        nc.vector.reciprocal(rec[:], s_red[:])
        att[p]["rec"][ci] = rec

    def tail_z_mul(p, ci, hi):
        """Pair tail for one (member, ci-half): the y-accumulator IS z (w_w
        folded into g on the host); normalize by the softmax denominator.
        No +zb bias: BatchNorm's mean subtraction absorbs the per-channel
        w_w@g_b + w_b shift exactly."""
        nq = PAIRS[p][ci]
        nsl = slice(nq * W, (nq + 1) * W)
        rec = att[p]["rec"][ci]
        nc.vector.tensor_mul(zsb[hi][:, nsl], att[p]["yps"][ci, hi][:], rec[:])

    def tail_z_stats(p, ci, hi):
        """BN partial sum / sumsq for one (member, ci-half) of z."""
        nq = PAIRS[p][ci]
        nsl = slice(nq * W, (nq + 1) * W)
        col = hi * 4 + nq
        nc.vector.reduce_sum(parts[:, col:col + 1], zsb[hi][:, nsl], axis=AX.X)
        sq = sp.tile([128, W], F32, name="sq_scr", tag="sq_scr", bufs=1)
        nc.scalar.activation(sq[:], zsb[hi][:, nsl], AF.Square,
                             accum_out=parts[:, 8 + col:9 + col])

    def pair_tails(p):
        # DVE order matters: the next pair's first y-matmuls hit banks in
        # (A0, B0, A1, B1) order, so free them (mul by reciprocal) in that
        # order before the slower BN-stats pieces.
        for ci in (0, 1):
            tail_a(p, ci)
        for hi in (0, 1):
            for ci in (0, 1):
                tail_z_mul(p, ci, hi)
        for ci in (0, 1):
            for hi in (0, 1):
                tail_z_stats(p, ci, hi)

    # --- emission: interleave projections into pair 0's m-loop; y-stream
    # SKEW blocks behind the f-stream; pair 0's tails at pair 1's head
    # (before its first y-allocation: bufs=1 PSUM WAR ordering) ---
    SKEW = 2
    seq = [(p, mb) for p in range(len(PAIRS)) for mb in range(MB)]
    state = {"fi": 0, "yi": 0}

    def emit_f():
        p, mb = seq[state["fi"]]
        if mb == 0:
            att_begin_f(p)
        att_f(p, mb)
        state["fi"] += 1

    def emit_y():
        p, mb = seq[state["yi"]]
        if mb == 0:
            if p > 0:
                pair_tails(p - 1)  # previous pair's tails free its banks
            att_begin_y(p)
        att_y(p, mb)
        state["yi"] += 1

    def fstep():
        emit_f()
        while state["fi"] - state["yi"] > SKEW:
            emit_y()

    # strip 0 projections, then the interleaved strip loop (pair 0's
    # f-stream + remaining strips + the previous body's deferred tails)
    for ho in (0, 1):
        proj_u(0, ho)
    for blk in range(MB_PER_STRIP):
        proj_g(0, blk)
    for s in range(1, N // SW):
        fetch_strip(s)
        if s + 1 < N // SW:
            fetch_strip(s + 1)  # prefetch next strip
        for st, fn in prev_steps or ():
            if st == s:
                fn()  # previous body's deferred tail piece
        # interleave the strip's PSUM allocations between f-emissions so the
        # shared psf ring's WAR partners (ACT u-epilogue, DVE g-copy) get
        # ~2 PE slots of drain time instead of clustering. f(4s) stays after
        # both u-halves (it reads this strip's phi blocks).
        proj_u(s, 0)
        proj_g(s, 0)
        fstep()
        proj_g(s, 1)
        fstep()
        proj_u(s, 1)
        proj_g(s, 2)
        fstep()
        proj_g(s, 3)
        fstep()
    while state["fi"] < len(seq):
        fstep()
    while state["yi"] < len(seq):
        emit_y()

    # --- everything after the last y is deferred into the NEXT body's
    # strip loop: last pair's tails + BN stats reduce + AllReduce at strip 1
    # (before the next body's first y-allocation), then the collective-
    # consuming affine/writeback late in the strip loop ---
    statsg = sp.tile([128, 4], F32, name="statsg", tag="statsg")
    last = len(PAIRS) - 1

    def bn_launch():
        pair_tails(last)
        stats = sp.tile([128, 4], F32, name="stats", tag="stats")
        nc.vector.reduce_sum(stats[:],
                             parts[:].rearrange("p (g c) -> p g c", c=4),
                             axis=AX.X)
        ar_in = dp.tile([128, 4], F32, name="ar_in", tag="ar_in")
        ar_out = dp.tile([128, 4], F32, name="ar_out", tag="ar_out",
                         addr_space="Shared")
        nc.sync.dma_start(ar_in[:], stats[:])
        nc.gpsimd.collective_compute(
            "AllReduce", mybir.AluOpType.add,
            replica_groups=[list(range(NCORES))],
            ins=[ar_in.opt()], outs=[ar_out.opt()],
        )
        nc.gpsimd.dma_start(statsg[:], ar_out[:])

    box = {}

    def tail_stats():
        # All on DVE: an ACT Sqrt/Ln here would force a 1.3us activation-
        # table swap in the middle of the next body's exp stream. rsqrt via
        # quake bit-trick + 2 Newton steps.
        inv_cnt = 1.0 / (B * N)
        moments = sp.tile([128, 4], F32, name="moments", tag="moments")
        nc.vector.tensor_scalar_mul(moments[:], statsg[:], inv_cnt)
        mean, ex2 = moments[:, 0:2], moments[:, 2:4]
        msq = sp.tile([128, 2], F32, name="msq", tag="msq")
        nc.vector.tensor_mul(msq[:], mean, mean)
        var = sp.tile([128, 2], F32, name="var", tag="var")
        nc.vector.tensor_sub(var[:], ex2, msq[:])
        vpe = sp.tile([128, 2], F32, name="vpe", tag="vpe")
        nc.vector.tensor_scalar_add(vpe[:], var[:], EPS)
        vh = sp.tile([128, 2], F32, name="vh", tag="vh")
        nc.vector.tensor_scalar_mul(vh[:], vpe[:], 0.5)
        I32 = mybir.dt.int32
        magic = sp.tile([128, 2], I32, name="magic", tag="magic")
        nc.vector.memset(magic[:], 0x5F3759DF)
        one_t = sp.tile([128, 2], I32, name="one_t", tag="one_t")
        nc.vector.memset(one_t[:], 1)
        rinv = sp.tile([128, 2], F32, name="rinv", tag="rinv")
        ri = rinv[:].bitcast(I32)
        nc.vector.tensor_tensor(ri, vpe[:].bitcast(I32), one_t[:],
                                mybir.AluOpType.logical_shift_right)
        nc.vector.tensor_tensor(ri, magic[:], ri, mybir.AluOpType.subtract)
        scr = sp.tile([128, 2], F32, name="scr", tag="scr")
        for _ in range(2):  # y *= 1.5 - 0.5*v*y^2
            nc.vector.tensor_mul(scr[:], rinv[:], rinv[:])
            nc.vector.tensor_mul(scr[:], scr[:], vh[:])
            nc.vector.tensor_scalar(scr[:], scr[:], -1.0, 1.5,
                                    mybir.AluOpType.mult, mybir.AluOpType.add)
            nc.vector.tensor_mul(rinv[:], rinv[:], scr[:])
        scl = sp.tile([128, 2], F32, name="scl", tag="scl")
        nc.vector.tensor_mul(scl[:], cst[:, 6:8], rinv[:])
        mscl = sp.tile([128, 2], F32, name="mscl", tag="mscl")
        nc.vector.tensor_mul(mscl[:], mean, scl[:])
        shf = sp.tile([128, 2], F32, name="shf", tag="shf")
        nc.vector.tensor_sub(shf[:], cst[:, 8:10], mscl[:])
        box["scl"], box["shf"] = scl, shf

    QTR = CHUNK // 4

    def make_affine(j):
        def step():
            scl, shf = box["scl"], box["shf"]
            jsl = slice(j * QTR, (j + 1) * QTR)
            nc.vector.tensor_scalar(zsb[0][:, jsl], zsb[0][:, jsl],
                                    scl[:, 0:1], shf[:, 0:1],
                                    mybir.AluOpType.mult, mybir.AluOpType.add)
            nc.scalar.activation(zsb[1][:, jsl], zsb[1][:, jsl], AF.Identity,
                                 bias=shf[:, 1:2], scale=scl[:, 1:2])
            nc.gpsimd.dma_start(out[0:128, jsl], zsb[0][:, jsl])
            nc.gpsimd.dma_start(out[128:256, jsl], zsb[1][:, jsl])
        return step

    return ([(1, bn_launch),
             (11, tail_stats)] +
            [(12 + j, make_affine(j)) for j in range(4)])


def _dedupe_ldweights(nc):
    """Remove InstLdweights that reload the PE array with weights identical
    to the previous (kept) load, with only matmuls in between. Validated
    bit-exact on HW: 16-bit matmuls are non-self-loading, so the array
    content persists across matmuls. Loads carrying semaphore waits are
    kept (their waits gate correctness)."""
    removed = 0
    for b in nc.m.functions[0].blocks:
        insts = b.instructions
        keep = []
        last_key = None
        for inst in insts:
            if str(getattr(inst, "engine", "")).endswith("PE"):
                if isinstance(inst, mybir.InstLdweights):
                    key = str(inst.ins[0])
                    s = str(inst.sync_info)
                    has_sync = "SyncWait(" in s or "SyncUpdate(" in s
                    if key == last_key and not has_sync:
                        removed += 1
                        continue
                    last_key = key
                elif not isinstance(inst, mybir.InstMatmult):
                    last_key = None
            keep.append(inst)
        if len(keep) != len(insts):
            b.instructions[:] = keep
    return removed


def build_nc(n_bodies=1):
    nc = bacc.Bacc("TRN2", target_bir_lowering=False, debug=False,
                   num_devices=NCORES)
    x = nc.dram_tensor("x", [C, N], FDT, kind="ExternalInput")
    wts = {p: nc.dram_tensor(f"w_{p}", [2, 128, CIP if p == "g" else CI],
                             FDT, kind="ExternalInput")
           for p in ("ph", "g")}
    consts = nc.dram_tensor("consts", [128, 10], F32, kind="ExternalInput")
    ones_in = nc.dram_tensor("ones_in", [128, 128], F32R, kind="ExternalInput")
    out = nc.dram_tensor("out", [CI, CHUNK], F32, kind="ExternalOutput")

    with tile.TileContext(nc) as tc:
        with (
            tc.tile_pool(name="wp", bufs=1) as wp,
            tc.tile_pool(name="xp", bufs=2) as xp,
            tc.tile_pool(name="pp", bufs=1) as pp,
            tc.tile_pool(name="ep", bufs=4) as ep,
            tc.tile_pool(name="sp", bufs=1) as sp,
            tc.tile_pool(name="psf", bufs=4, space="PSUM") as psf,
            tc.tile_pool(name="psa", bufs=1, space="PSUM") as psa,
            tc.tile_pool(name="dp", bufs=1, space="DRAM") as dp,
        ):
            pools = (wp, xp, pp, ep, sp, psf, psa, None, dp)
            tensors = (x, wts, consts, out, ones_in)
            steps = None
            for _ in range(n_bodies):
                steps = build_body(nc, tc, pools, tensors, prev_steps=steps)
            for _, step in steps:  # last body's deferred tail
                step()
    nc.compile()
    _dedupe_ldweights(nc)
    return nc


def make_in_maps(inputs):
    np16 = mybir.dt.np(FDT)
    x = np.asarray(inputs["x"], np.float32)
    theta_w = np.asarray(inputs["theta_w"], np.float64)
    phi_w = np.asarray(inputs["phi_w"], np.float64)
    theta_b = np.asarray(inputs["theta_b"], np.float64)
    # scores: f[n,m] = theta_x_n . phi_x_m = x_n.(M x_m) + v.x_m + c_n where
    # M = theta_w.T @ phi_w, v = phi_w.T @ theta_b; the n-only term c_n drops
    # out of the softmax over m. v rides as column CI of the g projection.
    M = theta_w.T @ phi_w
    v = phi_w.T @ theta_b
    # fold the z projection into g: z = w_w @ (sum_m g_m E_m) = sum_m
    # (w_w @ g_m) E_m, so the attention's y-accumulator produces z directly.
    g_proj = (np.asarray(inputs["w_w"], np.float64)
              @ np.asarray(inputs["g_w"], np.float64))
    g_ext = np.concatenate(
        [g_proj.T.astype(np.float32),
         v[:, None].astype(np.float32),
         np.zeros((C, 1), np.float32)], axis=1)  # (C, CI+2)
    wT = {
        "ph": np.ascontiguousarray(M.T.astype(np16).reshape(2, 128, CI)),
        "g": np.ascontiguousarray(g_ext.astype(np16).reshape(2, 128, CIP)),
    }
    # note: no z bias needed anywhere — BatchNorm's mean subtraction absorbs
    # the per-channel w_w@g_b + w_b shift exactly
    consts = np.zeros((128, 10), np.float32)
    for j, vv in enumerate((np.zeros(CI), np.zeros(CI), np.zeros(CI),
                            inputs["bn_gamma"], inputs["bn_beta"])):
        vv = np.asarray(vv, np.float32)
        consts[:, 2 * j] = vv[:128]
        consts[:, 2 * j + 1] = vv[128:]
    in_maps = []
    for k in range(NCORES):
        b, q = divmod(k, 4)
        xb = np.roll(x[b], -q * CHUNK, axis=1)
        in_maps.append({
            "x": np.ascontiguousarray(xb.astype(np16)),
            "w_ph": wT["ph"], "w_g": wT["g"],
            "consts": consts, "ones_in": np.ones((128, 128), np.float32),
        })
    return in_maps


def assemble(results):
    out = np.empty((B, CI, N), np.float32)
    for k in range(NCORES):
        b, q = divmod(k, 4)
        out[b, :, q * CHUNK:(q + 1) * CHUNK] = results[k]["out"]
    return out


_NC_CACHE = {}


def kernel(**inputs) -> np.ndarray:
    if "nc" not in _NC_CACHE:
        _NC_CACHE["nc"] = build_nc()
    nc = _NC_CACHE["nc"]
    in_maps = make_in_maps(inputs)
    res = run_bass_kernel_spmd(nc, in_maps, list(range(NCORES)))
    return assemble(res.results)


# revision 9
# speedup vs baseline: 1.4748x; 1.1797x over previous
"""Trainium2 Bass kernel for nn_NONLocal_Correlation (non-local block, B=2,
C=CI=256, N=8192).

Sharding: 8 cores = (batch b, query-chunk q) with b = core//4, q = core%4.
Each core computes out[b, :, q*2048:(q+1)*2048]. x[b] is passed rolled by
-q*2048 along n so the core's query columns are always x_rot[:, :2048];
m-order permutation is irrelevant (softmax sums over m).

Per-core algorithm — 16-bit matmul operands (fp16 for the score path, whose
error budget is tight; bf16 for the exp/value path, whose values need fp32
exponent range), fp32 PSUM accumulation throughout:
  - scores are algebraically reduced: f[n,m] = (th_w x_n + th_b).(ph_w x_m +
    ph_b) = x_n.(M x_m) + v.x_m + c_n with M = th_w.T @ ph_w and
    v = ph_w.T @ th_b host-precomputed; the n-only term c_n drops out of the
    softmax over m.  So only ONE projection u = M x is computed; v.x_m rides
    as column CI of the widened g projection and is extracted to an fp32
    per-m-block bias tile for the exp.  Other biases cost nothing: g_b/w_b
    shift z per-channel and cancel against BatchNorm's mean subtraction.
  - w_w is folded into the g weights on the host (z = w_w @ sum g E =
    sum (w_w g) E), so the attention's PSUM accumulator produces z directly.
  - x is DMA'd once (fp16, strip-wise) and stays SBUF-resident; it serves as
    the projections' operand AND the f-matmul's moving operand.
  - attention processes n-chunks in PAIRS (A, B) so each stationary operand
    serves two moving streams back-to-back; legalization emits an
    InstLdweights before every matmul, and `_dedupe_ldweights` removes the
    now-redundant consecutive identical loads post-compile (validated
    bit-exact on HW), halving the PE's weight-load overhead:
        f_A += u_blk.T x_A ; f_B += u_blk.T x_B      (ldw u0,[u0],u1,[u1])
        E_A = exp(f_A + bias_m); E_B likewise        (ACT, bf16 out)
        z_A += g_blk.T E_A ; z_B += g_blk.T E_B      (ldw g0,[g0],g1,[g1])
        s_acc_A += E_A ; s_acc_B += E_B              (DVE fp32 denominator)
    The y-stream runs SKEW=2 m-blocks behind the f-stream so the PE never
    stalls on ACT's exp.  PSUM: 4 y-accumulator banks (bufs=1, reused by the
    next pair after its tails) + the shared 4-bank matmul ring.
  - pair tails (ones-matmul denominator reduce, reciprocal, z *= rec, BN
    partials) are emitted at the next pair's head — before its first
    y-allocation so the bufs=1 PSUM WAR ordering holds.
  - BatchNorm (training stats over (b, n)): per-core partial sum/sumsq per
    channel, 2KB AllReduce across all 8 cores.  Everything after the last
    y-matmul is deferred into the NEXT replicated body's strip-1 emission so
    no in-order engine queue serializes the collective's latency; rsqrt is
    computed on DVE (bit-trick + Newton) to avoid ACT table swaps.
"""

import numpy as np

import concourse.bacc as bacc
import concourse.mybir as mybir
import concourse.tile as tile
from concourse.bass_utils import run_bass_kernel_spmd

B, C, N, CI = 2, 256, 8192, 256
CIP = CI + 2              # g-projection moving width: g channels + exp-bias col
                          # + pad (even free dim)
CHUNK = N // 4            # 2048 query rows per core
W = 512                   # moving free-dim width
MB = N // 128             # 64 m-blocks
SW = 512                  # x strip width for projections
MB_PER_STRIP = SW // 128  # 4
EPS = 1e-5
NCORES = 8

F32 = mybir.dt.float32
F32R = mybir.dt.float32r
FDT = mybir.dt.float16    # score-path operand dtype (x, u, score weights)
YDT = mybir.dt.bfloat16   # value-path operand dtype (E, g')
AF = mybir.ActivationFunctionType
AX = mybir.AxisListType

PAIRS = ((0, 1), (2, 3))  # n-chunk pairs sharing stationary loads


def build_body(nc, tc, pools, tensors, prev_steps=None):
    """Emit one full per-core computation. Separated so timing harnesses can
    replicate the body.

    Returns a list of deferred-tail closures (pair-1 tails + BN collective +
    affine/writeback) emitted inside the NEXT body's strip loop; see module
    docstring."""
    wp, xp, pp, ep, sp, psf, psa, psz, dp = pools
    x, wts, consts, out, ones_in = tensors

    # --- x halves, SBUF-resident, strip-wise DMA (strips 0-1 ahead of
    # weights in the sync queue; rest prefetched one strip ahead) ---
    xall = [xp.tile([128, N], FDT, name=f"xall{hi}", tag=f"xall{hi}")
            for hi in (0, 1)]
    fetched = {}

    def fetch_strip(s):
        if s in fetched:
            return
        fetched[s] = True
        for hi in (0, 1):
            nc.sync.dma_start(xall[hi][:, s * SW:(s + 1) * SW],
                              x[hi * 128:(hi + 1) * 128, s * SW:(s + 1) * SW])

    fetch_strip(0)
    fetch_strip(1)

    # --- weights / constants (u weights on the scalar queue so they don't
    # queue behind the x strips on sync) ---
    w_sb = {}
    for p in ("ph", "g"):
        for hi in (0, 1):
            wd = CIP if p == "g" else CI
            t = wp.tile([128, wd], FDT, name=f"w_{p}{hi}", tag=f"w_{p}{hi}")
            (nc.scalar if p == "ph" else nc.sync).dma_start(t[:], wts[p][hi])
            w_sb[p, hi] = t
    # bufs=2: the previous body's deferred BN tail reads its own cst while
    # this body's cst DMA lands.
    cst = wp.tile([128, 10], F32, name="cst", tag="cst", bufs=2)
    nc.scalar.dma_start(cst[:], consts[:])
    ones = wp.tile([128, 128], F32R, name="ones", tag="ones")
    nc.scalar.dma_start(ones[:], ones_in[:])

    # --- persistent activations ---
    # phi[hi] holds u = M x with M = theta_w.T @ phi_w (host-precomputed).
    phi = [pp.tile([128, N], FDT, name=f"phi{hi}", tag=f"phi{hi}") for hi in (0, 1)]
    gsb = pp.tile([128, MB * CIP], YDT, name="gsb", tag="gsb")
    biasb = pp.tile([128, MB], F32, name="biasb", tag="biasb")
    zsb = [pp.tile([128, CHUNK], F32, name=f"z{hi}", tag=f"z{hi}") for hi in (0, 1)]
    # per-(nq,hi) BN partials: cols hi*4+nq sum, 8 + hi*4+nq sumsq
    parts = sp.tile([128, 16], F32, name="parts", tag="parts")

    def proj_u(s, ho):
        fp = psf.tile([128, W], F32, name="mm_ps", tag="mm")
        nc.tensor.matmul(fp[:], w_sb["ph", 0][:, ho * 128:(ho + 1) * 128],
                         xall[0][:, s * SW:(s + 1) * SW], start=True, stop=False)
        nc.tensor.matmul(fp[:], w_sb["ph", 1][:, ho * 128:(ho + 1) * 128],
                         xall[1][:, s * SW:(s + 1) * SW], start=False, stop=True)
        nc.scalar.activation(phi[ho][:, s * SW:(s + 1) * SW], fp[:],
                             AF.Identity)

    def proj_g(s, blk):
        mb = s * MB_PER_STRIP + blk
        bsl = slice(s * SW + blk * 128, s * SW + (blk + 1) * 128)
        gp = psf.tile([128, CIP], F32, name="g_ps", tag="mm")
        nc.tensor.matmul(gp[:], xall[0][:, bsl], w_sb["g", 0][:],
                         start=True, stop=False)
        nc.tensor.matmul(gp[:], xall[1][:, bsl], w_sb["g", 1][:],
                         start=False, stop=True)
        nc.vector.tensor_copy(gsb[:, mb * CIP:(mb + 1) * CIP], gp[:])
        nc.vector.tensor_copy(biasb[:, mb:mb + 1], gp[:, CI:CI + 1])

    # attention state per chunk-pair. The f-stream only needs the E dict
    # (att_begin_f); the PSUM y-accumulators are allocated lazily at the
    # pair's first y-matmul (att_begin_y) so the previous pair's tails are
    # emitted first — the bufs=1 bank reuse WAR needs that order.
    att = {}

    def att_begin_f(p):
        att[p] = {"yps": None, "sacc": None, "rec": {}, "E": {}}

    def att_begin_y(p):
        att[p]["yps"] = {(ci, hi): psa.tile([128, W], F32,
                                            name=f"y_ps{ci}{hi}",
                                            tag=f"y_ps{ci}{hi}", bufs=1)
                         for ci in (0, 1) for hi in (0, 1)}
        att[p]["sacc"] = {ci: sp.tile([128, W], F32R, name=f"s_acc{ci}",
                                      tag=f"s_acc{ci}", bufs=2)
                          for ci in (0, 1)}

    def att_f(p, mb):
        """Score matmuls + exps for one m-block of both pair members.
        Emission order phi0:A, phi0:B, phi1:A, phi1:B makes the B-loads
        redundant for the post-compile ldweights dedupe."""
        msl = slice(mb * 128, (mb + 1) * 128)
        nsls = [slice(nq * W, (nq + 1) * W) for nq in PAIRS[p]]
        fps = [psf.tile([128, W], F32, name=f"f_ps{ci}", tag="mm")
               for ci in (0, 1)]
        for hi in (0, 1):
            for ci in (0, 1):
                nc.tensor.matmul(fps[ci][:], phi[hi][:, msl],
                                 xall[hi][:, nsls[ci]],
                                 start=(hi == 0), stop=(hi == 1))
        Es = []
        for ci in (0, 1):
            E = ep.tile([128, W], YDT, name=f"E{ci}", tag=f"E{ci}")
            nc.scalar.activation(E[:], fps[ci][:], AF.Exp,
                                 bias=biasb[:, mb:mb + 1])
            Es.append(E)
        att[p]["E"][mb] = Es

    def att_y(p, mb):
        """y-accumulate matmuls + denominator adds for one m-block, both
        pair members. Emitted SKEW m-blocks behind att_f. Order g0:A, g0:B,
        g1:A, g1:B for the ldweights dedupe."""
        yps, sacc = att[p]["yps"], att[p]["sacc"]
        Es = att[p]["E"].pop(mb)
        st, fin = (mb == 0), (mb == MB - 1)
        for hi in (0, 1):
            gs = gsb[:, mb * CIP + hi * 128:mb * CIP + (hi + 1) * 128]
            for ci in (0, 1):
                nc.tensor.matmul(yps[ci, hi][:], gs, Es[ci][:],
                                 start=st, stop=fin)
        for ci in (0, 1):
            if mb == 0:
                nc.vector.tensor_copy(sacc[ci][:], Es[ci][:])
            else:
                nc.vector.tensor_add(sacc[ci][:], sacc[ci][:], Es[ci][:])

    def tail_a(p, ci):
        """Pair tail part 1 for one member: reduce+broadcast the softmax
        denominators with a ones-matmul, then reciprocal."""
        sacc = att[p]["sacc"][ci]
        s_red = psf.tile([128, W], F32, name="s_red", tag="mm")
        nc.tensor.matmul(s_red[:], ones[:], sacc[:], start=True, stop=True)
        rec = sp.tile([128, W], F32, name="rec", tag="rec", bufs=2)
        nc.vector.reciprocal(rec[:], s_red[:])
        att[p]["rec"][ci] = rec

    def tail_z_mul(p, ci, hi):
        """Pair tail for one (member, ci-half): the y-accumulator IS z (w_w
        folded into g on the host); normalize by the softmax denominator.
        No +zb bias: BatchNorm's mean subtraction absorbs the per-channel
        w_w@g_b + w_b shift exactly."""
        nq = PAIRS[p][ci]
        nsl = slice(nq * W, (nq + 1) * W)
        rec = att[p]["rec"][ci]
        nc.vector.tensor_mul(zsb[hi][:, nsl], att[p]["yps"][ci, hi][:], rec[:])

    def tail_z_stats(p, ci, hi):
        """BN partial sum / sumsq for one (member, ci-half) of z."""
        nq = PAIRS[p][ci]
        nsl = slice(nq * W, (nq + 1) * W)
        col = hi * 4 + nq
        nc.vector.reduce_sum(parts[:, col:col + 1], zsb[hi][:, nsl], axis=AX.X)
        sq = sp.tile([128, W], F32, name="sq_scr", tag="sq_scr", bufs=1)
        nc.scalar.activation(sq[:], zsb[hi][:, nsl], AF.Square,
                             accum_out=parts[:, 8 + col:9 + col])

    def pair_tails(p):
        # DVE order matters: the next pair's first y-matmuls hit banks in
        # (A0, B0, A1, B1) order, so free them (mul by reciprocal) in that
        # order before the slower BN-stats pieces.
        for ci in (0, 1):
            tail_a(p, ci)
        for hi in (0, 1):
            for ci in (0, 1):
                tail_z_mul(p, ci, hi)
        for ci in (0, 1):
            for hi in (0, 1):
                tail_z_stats(p, ci, hi)

    # --- emission: interleave projections into pair 0's m-loop; y-stream
    # SKEW blocks behind the f-stream; pair 0's tails at pair 1's head
    # (before its first y-allocation: bufs=1 PSUM WAR ordering) ---
    SKEW = 2
    seq = [(p, mb) for p in range(len(PAIRS)) for mb in range(MB)]
    state = {"fi": 0, "yi": 0}

    def emit_f():
        p, mb = seq[state["fi"]]
        if mb == 0:
            att_begin_f(p)
        att_f(p, mb)
        state["fi"] += 1

    def emit_y():
        p, mb = seq[state["yi"]]
        if mb == 0:
            if p > 0:
                pair_tails(p - 1)  # previous pair's tails free its banks
            att_begin_y(p)
        att_y(p, mb)
        state["yi"] += 1

    def fstep():
        emit_f()
        while state["fi"] - state["yi"] > SKEW:
            emit_y()

    # strip 0 projections, then the interleaved strip loop (pair 0's
    # f-stream + remaining strips + the previous body's deferred tails)
    for ho in (0, 1):
        proj_u(0, ho)
    for blk in range(MB_PER_STRIP):
        proj_g(0, blk)
    for s in range(1, N // SW):
        fetch_strip(s)
        if s + 1 < N // SW:
            fetch_strip(s + 1)  # prefetch next strip
        for st, fn in prev_steps or ():
            if st == s:
                fn()  # previous body's deferred tail piece
        # interleave the strip's PSUM allocations between f-emissions so the
        # shared psf ring's WAR partners (ACT u-epilogue, DVE g-copy) get
        # ~2 PE slots of drain time instead of clustering. f(4s) stays after
        # both u-halves (it reads this strip's phi blocks).
        proj_u(s, 0)
        proj_g(s, 0)
        fstep()
        proj_g(s, 1)
        fstep()
        proj_u(s, 1)
        proj_g(s, 2)
        fstep()
        proj_g(s, 3)
        fstep()
    while state["fi"] < len(seq):
        fstep()
    while state["yi"] < len(seq):
        emit_y()

    # --- everything after the last y is deferred into the NEXT body's
    # strip loop: last pair's tails + BN stats reduce + AllReduce at strip 1
    # (before the next body's first y-allocation), then the collective-
    # consuming affine/writeback late in the strip loop ---
    statsg = sp.tile([128, 4], F32, name="statsg", tag="statsg")
    last = len(PAIRS) - 1

    def bn_launch():
        pair_tails(last)
        stats = sp.tile([128, 4], F32, name="stats", tag="stats")
        nc.vector.reduce_sum(stats[:],
                             parts[:].rearrange("p (g c) -> p g c", c=4),
                             axis=AX.X)
        ar_in = dp.tile([128, 4], F32, name="ar_in", tag="ar_in")
        ar_out = dp.tile([128, 4], F32, name="ar_out", tag="ar_out",
                         addr_space="Shared")
        nc.sync.dma_start(ar_in[:], stats[:])
        nc.gpsimd.collective_compute(
            "AllReduce", mybir.AluOpType.add,
            replica_groups=[list(range(NCORES))],
            ins=[ar_in.opt()], outs=[ar_out.opt()],
        )
        nc.gpsimd.dma_start(statsg[:], ar_out[:])

    box = {}

    def tail_stats():
        # All on DVE: an ACT Sqrt/Ln here would force a 1.3us activation-
        # table swap in the middle of the next body's exp stream. rsqrt via
        # quake bit-trick + 2 Newton steps.
        inv_cnt = 1.0 / (B * N)
        moments = sp.tile([128, 4], F32, name="moments", tag="moments")
        nc.vector.tensor_scalar_mul(moments[:], statsg[:], inv_cnt)
        mean, ex2 = moments[:, 0:2], moments[:, 2:4]
        msq = sp.tile([128, 2], F32, name="msq", tag="msq")
        nc.vector.tensor_mul(msq[:], mean, mean)
        var = sp.tile([128, 2], F32, name="var", tag="var")
        nc.vector.tensor_sub(var[:], ex2, msq[:])
        vpe = sp.tile([128, 2], F32, name="vpe", tag="vpe")
        nc.vector.tensor_scalar_add(vpe[:], var[:], EPS)
        vh = sp.tile([128, 2], F32, name="vh", tag="vh")
        nc.vector.tensor_scalar_mul(vh[:], vpe[:], 0.5)
        I32 = mybir.dt.int32
        magic = sp.tile([128, 2], I32, name="magic", tag="magic")
        nc.vector.memset(magic[:], 0x5F3759DF)
        one_t = sp.tile([128, 2], I32, name="one_t", tag="one_t")
        nc.vector.memset(one_t[:], 1)
        rinv = sp.tile([128, 2], F32, name="rinv", tag="rinv")
        ri = rinv[:].bitcast(I32)
        nc.vector.tensor_tensor(ri, vpe[:].bitcast(I32), one_t[:],
                                mybir.AluOpType.logical_shift_right)
        nc.vector.tensor_tensor(ri, magic[:], ri, mybir.AluOpType.subtract)
        scr = sp.tile([128, 2], F32, name="scr", tag="scr")
        for _ in range(2):  # y *= 1.5 - 0.5*v*y^2
            nc.vector.tensor_mul(scr[:], rinv[:], rinv[:])
            nc.vector.tensor_mul(scr[:], scr[:], vh[:])
            nc.vector.tensor_scalar(scr[:], scr[:], -1.0, 1.5,
                                    mybir.AluOpType.mult, mybir.AluOpType.add)
            nc.vector.tensor_mul(rinv[:], rinv[:], scr[:])
        scl = sp.tile([128, 2], F32, name="scl", tag="scl")
        nc.vector.tensor_mul(scl[:], cst[:, 6:8], rinv[:])
        mscl = sp.tile([128, 2], F32, name="mscl", tag="mscl")
        nc.vector.tensor_mul(mscl[:], mean, scl[:])
        shf = sp.tile([128, 2], F32, name="shf", tag="shf")
        nc.vector.tensor_sub(shf[:], cst[:, 8:10], mscl[:])
        box["scl"], box["shf"] = scl, shf

    QTR = CHUNK // 4

    def make_affine(j):
        def step():
            scl, shf = box["scl"], box["shf"]
            jsl = slice(j * QTR, (j + 1) * QTR)
            nc.vector.tensor_scalar(zsb[0][:, jsl], zsb[0][:, jsl],
                                    scl[:, 0:1], shf[:, 0:1],
                                    mybir.AluOpType.mult, mybir.AluOpType.add)
            nc.scalar.activation(zsb[1][:, jsl], zsb[1][:, jsl], AF.Identity,
                                 bias=shf[:, 1:2], scale=scl[:, 1:2])
            nc.gpsimd.dma_start(out[0:128, jsl], zsb[0][:, jsl])
            nc.gpsimd.dma_start(out[128:256, jsl], zsb[1][:, jsl])
        return step

    return ([(1, bn_launch),
             (11, tail_stats)] +
            [(12 + j, make_affine(j)) for j in range(4)])


def _dedupe_ldweights(nc):
    """Remove InstLdweights that reload the PE array with weights identical
    to the previous (kept) load, with only matmuls in between. Validated
    bit-exact on HW: 16-bit matmuls are non-self-loading, so the array
    content persists across matmuls. Loads carrying semaphore waits are
    kept (their waits gate correctness)."""
    removed = 0
    for b in nc.m.functions[0].blocks:
        insts = b.instructions
        keep = []
        last_key = None
        for inst in insts:
            if str(getattr(inst, "engine", "")).endswith("PE"):
                if isinstance(inst, mybir.InstLdweights):
                    key = str(inst.ins[0])
                    s = str(inst.sync_info)
                    has_sync = "SyncWait(" in s or "SyncUpdate(" in s
                    if key == last_key and not has_sync:
                        removed += 1
                        continue
                    last_key = key
                elif not isinstance(inst, mybir.InstMatmult):
                    last_key = None
            keep.append(inst)
        if len(keep) != len(insts):
            b.instructions[:] = keep
    return removed


def build_nc(n_bodies=1):
    nc = bacc.Bacc("TRN2", target_bir_lowering=False, debug=False,
                   num_devices=NCORES)
    x = nc.dram_tensor("x", [C, N], FDT, kind="ExternalInput")
    wts = {p: nc.dram_tensor(f"w_{p}", [2, 128, CIP if p == "g" else CI],
                             FDT, kind="ExternalInput")
           for p in ("ph", "g")}
    consts = nc.dram_tensor("consts", [128, 10], F32, kind="ExternalInput")
    ones_in = nc.dram_tensor("ones_in", [128, 128], F32R, kind="ExternalInput")
    out = nc.dram_tensor("out", [CI, CHUNK], F32, kind="ExternalOutput")

    with tile.TileContext(nc) as tc:
        with (
            tc.tile_pool(name="wp", bufs=1) as wp,
            tc.tile_pool(name="xp", bufs=2) as xp,
            tc.tile_pool(name="pp", bufs=1) as pp,
            tc.tile_pool(name="ep", bufs=4) as ep,
            tc.tile_pool(name="sp", bufs=1) as sp,
            tc.tile_pool(name="psf", bufs=4, space="PSUM") as psf,
            tc.tile_pool(name="psa", bufs=1, space="PSUM") as psa,
            tc.tile_pool(name="dp", bufs=1, space="DRAM") as dp,
        ):
            pools = (wp, xp, pp, ep, sp, psf, psa, None, dp)
            tensors = (x, wts, consts, out, ones_in)
            steps = None
            for _ in range(n_bodies):
                steps = build_body(nc, tc, pools, tensors, prev_steps=steps)
            for _, step in steps:  # last body's deferred tail
                step()
    nc.compile()
    _dedupe_ldweights(nc)
    return nc


def make_in_maps(inputs):
    np16 = mybir.dt.np(FDT)
    x = np.asarray(inputs["x"], np.float32)
    theta_w = np.asarray(inputs["theta_w"], np.float64)
    phi_w = np.asarray(inputs["phi_w"], np.float64)
    theta_b = np.asarray(inputs["theta_b"], np.float64)
    # scores: f[n,m] = theta_x_n . phi_x_m = x_n.(M x_m) + v.x_m + c_n where
    # M = theta_w.T @ phi_w, v = phi_w.T @ theta_b; the n-only term c_n drops
    # out of the softmax over m. v rides as column CI of the g projection.
    M = theta_w.T @ phi_w
    v = phi_w.T @ theta_b
    # fold the z projection into g: z = w_w @ (sum_m g_m E_m) = sum_m
    # (w_w @ g_m) E_m, so the attention's y-accumulator produces z directly.
    g_proj = (np.asarray(inputs["w_w"], np.float64)
              @ np.asarray(inputs["g_w"], np.float64))
    g_ext = np.concatenate(
        [g_proj.T.astype(np.float32),
         v[:, None].astype(np.float32),
         np.zeros((C, 1), np.float32)], axis=1)  # (C, CI+2)
    wT = {
        "ph": np.ascontiguousarray(M.T.astype(np16).reshape(2, 128, CI)),
        "g": np.ascontiguousarray(g_ext.astype(np16).reshape(2, 128, CIP)),
    }
    # note: no z bias needed anywhere — BatchNorm's mean subtraction absorbs
    # the per-channel w_w@g_b + w_b shift exactly
    consts = np.zeros((128, 10), np.float32)
    for j, vv in enumerate((np.zeros(CI), np.zeros(CI), np.zeros(CI),
                            inputs["bn_gamma"], inputs["bn_beta"])):
        vv = np.asarray(vv, np.float32)
        consts[:, 2 * j] = vv[:128]
        consts[:, 2 * j + 1] = vv[128:]
    in_maps = []
    for k in range(NCORES):
        b, q = divmod(k, 4)
        xb = np.roll(x[b], -q * CHUNK, axis=1)
        in_maps.append({
            "x": np.ascontiguousarray(xb.astype(np16)),
            "w_ph": wT["ph"], "w_g": wT["g"],
            "consts": consts, "ones_in": np.ones((128, 128), np.float32),
        })
    return in_maps


def assemble(results):
    out = np.empty((B, CI, N), np.float32)
    for k in range(NCORES):
        b, q = divmod(k, 4)
        out[b, :, q * CHUNK:(q + 1) * CHUNK] = results[k]["out"]
    return out


_NC_CACHE = {}


def kernel(**inputs) -> np.ndarray:
    if "nc" not in _NC_CACHE:
        _NC_CACHE["nc"] = build_nc()
    nc = _NC_CACHE["nc"]
    in_maps = make_in_maps(inputs)
    res = run_bass_kernel_spmd(nc, in_maps, list(range(NCORES)))
    return assemble(res.results)
